# revision 44
# baseline (speedup 1.0000x reference)
import sys

sys.path.insert(0, "/opt/trn_rl_repo")

import hashlib

import ml_dtypes
import numpy as np

import concourse.bass as bass
import concourse.mybir as mybir
import concourse.tile as tile
from concourse.library_config import mlp
from concourse.vector_clock import ScopedClock

dt = mybir.dt
AF = mybir.ActivationFunctionType
ALU = mybir.AluOpType

N_NODES = 100000
F_IN = 128
N_CLASSES = 40
NCORES = 8
NSH = 12500
NT = 98
NSHP = NT * 128  # 12544
NTG = NCORES * NT  # 784 global node tiles
QROWS = 2 * NSHP  # 25088 rows per int16-indexable quarter (= 2 cores)
NQ = 4
KCAP = 8  # max 128-idx chunks per dma_gather call (SWDGE ring capacity)
SINGLE_PACKET = False  # spread gather descriptors across packets


class PatchedTileContext(tile.TileContext):
    # walrus CoreV3 codegen accepts at most 1 sem wait on most instruction
    # structs; spread the final-drain waits over 1-wait nops.
    def _drain_and_barrier(self, tick_clock, wait_clock):
        collector = self.nc.sync.nop(nofuse=True)
        wait_clock.add_sem_waits(
            collector.ins, ScopedClock({None: tick_clock.global_clock})
        )
        si = collector.ins.sync_info
        waits = list(si.on_wait) if si and si.on_wait else []
        if len(waits) > 1:
            si.on_wait = waits[:1]
            for w in waits[1:]:
                extra = self.nc.sync.nop(nofuse=True)
                extra.ins.sync_info = mybir.SyncInfo(on_wait=[w], on_update=[])
        self.nc.sync.drain()
        self.nc.all_engine_barrier()
        assert self.sems is not None
        popped = self.nc._tile_sem_poison_stack.pop()
        assert popped is self._sem_poison
        self.nc.clear_and_free_semaphores(list(self.sems.allocated().values()))
        self.nc.all_engine_barrier()


def _split_excess_waits(nc, max_waits=1):
    # Same walrus limit for ordinary instructions: move excess sem waits onto
    # single-wait carrier instructions on the same engine, inserted just
    # before (per-engine order makes the stall equivalent).
    cnt = 0
    for f in nc.m.functions:
        for bb in f.blocks:
            insns = bb.instructions
            i = 0
            while i < len(insns):
                ins = insns[i]
                si = getattr(ins, "sync_info", None)
                waits = list(si.on_wait) if si is not None and si.on_wait else []
                if len(waits) > max_waits:
                    si.on_wait = waits[:1]
                    for w in waits[1:]:
                        if ins.engine == mybir.EngineType.Pool:
                            nop = mybir.InstEventSemaphore(
                                name=f"waitsplit_{cnt}", ins=[], outs=[]
                            )
                        else:
                            nop = mybir.InstNoOp(
                                name=f"waitsplit_{cnt}", ins=[], outs=[]
                            )
                        cnt += 1
                        nop.engine = ins.engine
                        nop.sync_info = mybir.SyncInfo(on_wait=[w], on_update=[])
                        insns.insert(i, nop)
                        i += 1
                i += 1
    return cnt


def _preprocess(edge_index):
    """Structure-only preprocessing (depends on edge_index, not on x/W).

    Layer tables are stored partition-major: row of node (g, p) lives at
    flat index p*196 + (g % 196) within its quarter's table; the AllGather
    table row for (core c, tile t, slot p) is ((c*128 + p)*NT + t) so the
    kernel can publish [128, NT, 128] tiles with identity-layout DMAs.
    """
    src = np.asarray(edge_index[0], dtype=np.int64)
    dst = np.asarray(edge_index[1], dtype=np.int64)
    deg = np.bincount(dst, minlength=N_NODES).astype(np.float32) + 1.0
    dinv = (1.0 / np.sqrt(deg)).astype(np.float32)
    loops = np.arange(N_NODES, dtype=np.int64)
    src = np.concatenate([src, loops])
    dst = np.concatenate([dst, loops])

    # per-edge source coordinates (global tile g, slot ps)
    s_core = src // NSH
    s_loc = src % NSH
    g = s_core * NT + (s_loc >> 7)
    ps = s_loc & 127
    q_of_edge = g // (2 * NT)
    # layer-1 table index (partition-major within quarter)
    lidx1 = ps * 196 + (g % 196)
    # layer-2 table index within quarter piece
    lidx2 = ((s_core - 2 * q_of_edge) * 128 + ps) * NT + (g % NT)

    core_of = dst // NSH
    per_core = []
    counts = np.zeros((NCORES, NT, NQ), np.int64)
    for c in range(NCORES):
        m = core_of == c
        ed = dst[m] - c * NSH
        t = ed >> 7
        slot = ed & 127
        q = q_of_edge[m]
        l1 = lidx1[m]
        l2 = lidx2[m]
        key = t * NQ + q  # sort by (t, q)
        order = np.argsort(key, kind="stable")
        per_core.append((t[order], q[order], slot[order], l1[order], l2[order]))
        np.add.at(counts[c], (t[order], q[order]), 1)

    K = np.ceil(counts / 128.0).astype(np.int64).max(axis=0)  # [NT, NQ]
    # chunk layout: for t: for q: K[t, q] chunks (tile's chunks contiguous)
    chunk_base = np.zeros((NT, NQ), np.int64)
    off = 0
    for t in range(NT):
        for q in range(NQ):
            chunk_base[t, q] = off
            off += K[t, q]
    nch_tot = off
    nchf = int(K.sum(axis=1).max())
    tot = nch_tot * 128

    idx1_ws, idx2_ws, dst_ws = [], [], []
    for c in range(NCORES):
        t, q, slot, l1, l2 = per_core[c]
        # rank of each edge within its (t, q) bucket (edges sorted by (t, q))
        key = t * NQ + q
        uniq, first, inv = np.unique(key, return_index=True, return_inverse=True)
        rank = np.arange(len(t)) - first[inv]
        pos = chunk_base[t, q] * 128 + rank
        idx1_flat = np.zeros(tot, np.int16)
        idx2_flat = np.zeros(tot, np.int16)
        dst_flat = np.full(tot, 999.0, np.float32)
        idx1_flat[pos] = l1.astype(np.int16)
        idx2_flat[pos] = l2.astype(np.int16)
        dst_flat[pos] = slot.astype(np.float32)
        # [128, tot//16]: 16-row wrap replicated 8x (one copy per gpsimd core)
        idx1_ws.append(
            np.ascontiguousarray(np.tile(idx1_flat.reshape(tot // 16, 16).T, (8, 1)))
        )
        idx2_ws.append(
            np.ascontiguousarray(np.tile(idx2_flat.reshape(tot // 16, 16).T, (8, 1)))
        )
        dst_ws.append(
            np.ascontiguousarray(
                dst_flat.reshape(nch_tot, 128).T
            )
        )

    iota = np.tile(np.arange(128, dtype=np.float32)[None, :], (128, nchf))
    iota = np.ascontiguousarray(
        iota.reshape(128, nchf, 128)
    )

    # dinv lookups: dinva[p, g] = dinv of node (g, p) (0 on padding)
    gg, pp = np.meshgrid(np.arange(NTG), np.arange(128), indexing="xy")
    node = (gg // NT) * NSH + (gg % NT) * 128 + pp  # [128, NTG]
    valid = ((gg % NT) * 128 + pp) < NSH
    dinva = np.where(valid, dinv[np.minimum(node, N_NODES - 1)], 0.0).astype(
        np.float32
    )
    dinv_ts = [np.ascontiguousarray(dinva[:, c * NT : (c + 1) * NT]) for c in range(NCORES)]

    return dict(
        K=K,
        chunk_base=chunk_base,
        nch_tot=nch_tot,
        nchf=nchf,
        tot=tot,
        idx1_ws=idx1_ws,
        idx2_ws=idx2_ws,
        dst_ws=dst_ws,
        iota=iota,
        dinva=np.ascontiguousarray(dinva),
        dinv_ts=dinv_ts,
    )


def _build(meta, ablate=()):
    K = meta["K"]
    chunk_base = meta["chunk_base"]
    nch_tot = meta["nch_tot"]
    nchf = meta["nchf"]

    nc = bass.Bass(num_devices=NCORES, num_swdge_queues=4)
    xs_in = nc.dram_tensor(
        "xs_t", [NQ, QROWS, F_IN], dt.bfloat16, kind="ExternalInput"
    )
    w1_in = nc.dram_tensor("w1", [F_IN, F_IN], dt.float32, kind="ExternalInput")
    w2_in = nc.dram_tensor("w2", [F_IN, 64], dt.float32, kind="ExternalInput")
    dinvt_in = nc.dram_tensor("dinvt", [128, NT], dt.float32, kind="ExternalInput")
    idx1_in = nc.dram_tensor("idx1", [128, nch_tot * 8], dt.int16, kind="ExternalInput")
    idx2_in = nc.dram_tensor("idx2", [128, nch_tot * 8], dt.int16, kind="ExternalInput")
    dst_in = nc.dram_tensor("dst_w", [128, nch_tot], dt.float32, kind="ExternalInput")
    iota_in = nc.dram_tensor(
        "iota_r", [128, nchf, 128], dt.float32, kind="ExternalInput"
    )
    out_t = nc.dram_tensor(
        "out_s", [128, NT, N_CLASSES], dt.float32, kind="ExternalOutput"
    )

    with PatchedTileContext(nc) as tc:
        with (
            tc.tile_pool(name="sbuf", bufs=1) as pool,
            tc.tile_pool(name="psum", bufs=1, space="PSUM") as psum,
            tc.tile_pool(name="dram", bufs=1, space="DRAM") as dram,
        ):
            w1_t = pool.tile([F_IN, F_IN], dt.float32)
            w2_t = pool.tile([F_IN, 64], dt.float32)
            dinvt_t = pool.tile([128, NT], dt.float32)
            # one idx buffer, re-filled between layers (WAR dep handled by Tile)
            idx_t = pool.tile([128, nch_tot * 8], dt.int16)
            dst_t = pool.tile([128, nch_tot], dt.float32)
            iota_t = pool.tile([128, nchf, 128], dt.float32)
            z_all = pool.tile([128, NT, N_CLASSES], dt.float32)
            zs_all = pool.tile([128, NT, N_CLASSES], dt.float32)
            nc.gpsimd.load_library(mlp)
            for d_ap, s_ap in [
                (w1_t, w1_in),
                (w2_t, w2_in),
                (dinvt_t, dinvt_in),
                (idx_t, idx1_in),
                (dst_t, dst_in),
                (iota_t, iota_in),
            ]:
                nc.sync.dma_start(d_ap[:], s_ap[:])

            bounce = dram.tile([128, NT, F_IN], dt.bfloat16)
            table2 = dram.tile(
                [NCORES * 128, NT, F_IN], dt.bfloat16, addr_space="Shared"
            )

            # gpsimd registers are scarce: one per distinct idx count, reused
            reg_cache = {}

            def nreg(v):
                if v not in reg_cache:
                    reg_cache[v] = nc.gpsimd.to_reg(v)
                return reg_cache[v]

            def aggregate(idx_t, tables, scope, tail):
                """Per dst tile t: acc[t] = sum over edges of table[src]^T,
                accumulated in PSUM across all 4 quarters, then tail(t, accs).

                tables[q] is an AP-able of shape [QROWS, 128] (bf16 rows).
                """
                _sid, _ = nc.enter_named_scope(scope, False)
                for t in range(NT):
                    o = int(chunk_base[t, 0])
                    ntot = int(K[t].sum())
                    assert ntot > 0
                    sel_t = pool.tile(
                        [128, nchf, 128], dt.bfloat16, name="sel", bufs=2
                    )
                    nc.vector.tensor_tensor(
                        out=sel_t[:, 0:ntot, :],
                        in0=dst_t[:, o : o + ntot].to_broadcast([128, ntot, 128]),
                        in1=iota_t[:, 0:ntot, :],
                        op=ALU.is_equal,
                    )
                    acc = psum.tile([128, 128], dt.float32, name="acc", bufs=2)
                    done = 0
                    for q in range(NQ):
                        kb = int(K[t, q])
                        ob = int(chunk_base[t, q])
                        for p0 in range(0, kb, KCAP):
                            kp = min(KCAP, kb - p0)
                            g_t = pool.tile(
                                [128, KCAP, 128], dt.bfloat16, name=f"g{q}", bufs=3
                            )
                            if "gather" not in ablate:
                                nc.gpsimd.dma_gather(
                                    g_t[:, 0:kp, :],
                                    tables[q],
                                    idx_t[:, (ob + p0) * 8 : (ob + p0 + kp) * 8],
                                    num_idxs=kp * 128,
                                    num_idxs_reg=nreg(kp * 128),
                                    elem_size=F_IN,
                                    queue_num=q,
                                    single_packet=SINGLE_PACKET,
                                )
                            elif "pe" not in ablate:
                                nc.vector.memset(g_t[:, 0:kp, :], 0.0)
                            cb = ob + p0 - o
                            for k in range(kp):
                                if "pe" in ablate:
                                    if done == 0:
                                        nc.vector.memset(acc[:], 0.0)
                                    done = ntot
                                    break
                                nc.tensor.matmul(
                                    acc[:],
                                    lhsT=g_t[:, k, :],
                                    rhs=sel_t[:, cb + k, :],
                                    start=(done == 0),
                                    stop=(done == ntot - 1),
                                )
                                done += 1
                    accs = pool.tile([128, 128], dt.float32, name="accs", bufs=3)
                    nc.scalar.copy(accs[:], acc[:])
                    tail(t, accs)
                nc.leave_named_scope(scope, _sid, False)

            # ---- layer 1
            def l1_tail(t, accs):
                h1p = psum.tile([128, 128], dt.float32, name="mm", bufs=2)
                nc.tensor.matmul(
                    h1p[:], lhsT=accs[:], rhs=w1_t[:], start=True, stop=True
                )
                a1 = pool.tile([128, 128], dt.float32, name="a1", bufs=2)
                nc.scalar.activation(
                    a1[:], h1p[:], AF.Relu, bias=0.0, scale=dinvt_t[:, t : t + 1]
                )
                p16 = pool.tile([128, 128], dt.bfloat16, name="p16", bufs=3)
                nc.scalar.activation(
                    p16[:], a1[:], AF.Copy, bias=0.0, scale=dinvt_t[:, t : t + 1]
                )
                nc.sync.dma_start(bounce[:, t, :], p16[:])

            aggregate(idx_t, [xs_in[q] for q in range(NQ)], "agg1", l1_tail)

            _sidG, _ = nc.enter_named_scope("ag", False)
            if "cc" not in ablate:
                nc.gpsimd.collective_compute(
                    "AllGather",
                    ALU.bypass,
                    replica_groups=[list(range(NCORES))],
                    ins=[bounce.opt()],
                    outs=[table2.opt()],
                )
            nc.leave_named_scope("ag", _sidG, False)

            # ---- layer 2
            nc.sync.dma_start(idx_t[:], idx2_in[:])
            t2 = table2[:].flatten_outer_dims()

            def l2_tail(t, accs):
                zp = psum.tile([128, 64], dt.float32, name="zp", bufs=2)
                nc.tensor.matmul(
                    zp[:], lhsT=accs[:], rhs=w2_t[:], start=True, stop=True
                )
                nc.scalar.activation(
                    z_all[:, t, :],
                    zp[:, 0:N_CLASSES],
                    AF.Copy,
                    bias=0.0,
                    scale=dinvt_t[:, t : t + 1],
                )

            aggregate(
                idx_t,
                [t2[q * QROWS : (q + 1) * QROWS, :] for q in range(NQ)],
                "agg2",
                l2_tail,
            )
            _sid2, _ = nc.enter_named_scope("l2tail", False)
            # batched log_softmax over classes
            mx = pool.tile([128, NT, 1], dt.float32, name="mx")
            nc.vector.tensor_reduce(mx[:], z_all[:], mybir.AxisListType.X, ALU.max)
            nc.vector.tensor_tensor(
                out=zs_all[:],
                in0=z_all[:],
                in1=mx[:, :, 0].to_broadcast([128, NT, N_CLASSES]),
                op=ALU.subtract,
            )
            nc.scalar.activation(z_all[:], zs_all[:], AF.Exp, bias=0.0, scale=1.0)
            sm = pool.tile([128, NT, 1], dt.float32, name="sm")
            nc.vector.tensor_reduce(sm[:], z_all[:], mybir.AxisListType.X, ALU.add)
            ls = pool.tile([128, NT, 1], dt.float32, name="ls")
            nc.scalar.activation(ls[:], sm[:], AF.Ln, bias=0.0, scale=1.0)
            nc.vector.tensor_tensor(
                out=z_all[:],
                in0=zs_all[:],
                in1=ls[:, :, 0].to_broadcast([128, NT, N_CLASSES]),
                op=ALU.subtract,
            )
            nc.sync.dma_start(out_t[:], z_all[:])
            nc.leave_named_scope("l2tail", _sid2, False)

    _split_excess_waits(nc)
    mybir.codegen_inst_isa_subclasses(nc)
    return nc


def _make_runner(nc):
    import jax
    from jax.sharding import Mesh, PartitionSpec

    try:
        from jax.experimental.shard_map import shard_map
    except ImportError:
        from jax.shard_map import shard_map

    from concourse.bass2jax import (
        _bass_exec_p,
        install_neuronx_cc_hook,
        partition_id_tensor,
    )

    install_neuronx_cc_hook()
    assert nc.dbg_addr is None
    partition_name = nc.partition_id_tensor.name if nc.partition_id_tensor else None

    in_names, out_names, out_avals = [], [], []
    for alloc in nc.m.functions[0].allocations:
        if not isinstance(alloc, mybir.MemoryLocationSet):
            continue
        name = alloc.memorylocations[0].name
        if alloc.kind == "ExternalInput":
            if name != partition_name:
                in_names.append(name)
        elif alloc.kind == "ExternalOutput":
            out_names.append(name)
            shape = tuple(alloc.tensor_shape)
            dtype = mybir.dt.np(alloc.dtype)
            out_avals.append(jax.core.ShapedArray(shape, dtype))
    n_params = len(in_names)
    n_outs = len(out_avals)
    all_names = in_names + out_names
    if partition_name is not None:
        all_names = all_names + [partition_name]
    donate = tuple(range(n_params, n_params + n_outs))

    def _body(*args):
        operands = list(args)
        if partition_name is not None:
            operands.append(partition_id_tensor())
        outs = _bass_exec_p.bind(
            *operands,
            out_avals=tuple(out_avals),
            in_names=tuple(all_names),
            out_names=tuple(out_names),
            lowering_input_output_aliases=(),
            sim_require_finite=True,
            sim_require_nnan=True,
            nc=nc,
        )
        return tuple(outs)

    devices = jax.devices()[:NCORES]
    mesh = Mesh(np.asarray(devices), ("core",))
    in_specs = (PartitionSpec("core"),) * (n_params + n_outs)
    out_specs = (PartitionSpec("core"),) * n_outs
    sharded = jax.jit(
        shard_map(
            _body, mesh=mesh, in_specs=in_specs, out_specs=out_specs, check_rep=False
        ),
        donate_argnums=donate,
        keep_unused=True,
    )

    state = {"dev_in": None, "dev_key": None}

    def run(in_maps):
        per_core = [[np.asarray(m[name]) for name in in_names] for m in in_maps]
        concat_in = [
            np.concatenate([per_core[c][i] for c in range(NCORES)], axis=0)
            for i in range(n_params)
        ]
        hkey = hashlib.sha1()
        for a in concat_in:
            hkey.update(a.tobytes())
        hkey = hkey.hexdigest()
        if state["dev_key"] != hkey:
            from jax.sharding import NamedSharding

            state["dev_in"] = [
                jax.device_put(a, NamedSharding(mesh, PartitionSpec("core")))
                for a in concat_in
            ]
            state["dev_key"] = hkey
        concat_zeros = [
            np.zeros((NCORES * a.shape[0], *a.shape[1:]), a.dtype) for a in out_avals
        ]
        out_arrs = sharded(*state["dev_in"], *concat_zeros)
        jax.block_until_ready(out_arrs)
        return [
            [
                np.asarray(out_arrs[i]).reshape(NCORES, *out_avals[i].shape)[c]
                for i in range(n_outs)
            ]
            for c in range(NCORES)
        ]

    run.sharded = sharded
    run.state = state
    run.mesh = mesh
    run.out_avals = out_avals
    run.body = _body
    run.n_params = n_params
    return run


_CACHE = {}


def kernel(**inputs):
    x = np.asarray(inputs["x"], np.float32)
    ei = np.asarray(inputs["edge_index"])
    W1 = np.asarray(inputs["W1"], np.float32)
    W2 = np.asarray(inputs["W2"], np.float32)
    b1 = np.asarray(inputs["b1"], np.float32)
    b2 = np.asarray(inputs["b2"], np.float32)
    assert not b1.any() and not b2.any(), "nonzero biases not supported"

    key = hashlib.sha1(ei.tobytes()).hexdigest()
    st = _CACHE.get(key)
    if st is None:
        meta = _preprocess(ei)
        nc = _build(meta)
        runner = _make_runner(nc)
        st = {"meta": meta, "runner": runner}
        _CACHE.clear()
        _CACHE[key] = st
    meta = st["meta"]

    # layer-1 gather table: xs[q, p*196 + gl, :] = dinv[n] * x[n] (bf16)
    # for node n of global tile g = q*196 + gl, slot p (zeros on padding)
    xpad = np.zeros((NCORES, NT * 128, F_IN), np.float32)
    for c in range(NCORES):
        xpad[c, :NSH] = meta["dinva"].T.reshape(NTG * 128, 1)[
            c * NSHP : c * NSHP + NSH
        ] * x[c * NSH : (c + 1) * NSH]
    xs_t = np.ascontiguousarray(
        xpad.reshape(NQ, 196, 128, F_IN)
        .transpose(0, 2, 1, 3)
        .reshape(NQ, QROWS, F_IN)
        .astype(ml_dtypes.bfloat16)
    )

    w2p = np.zeros((F_IN, 64), np.float32)
    w2p[:, :N_CLASSES] = W2
    in_maps = []
    for c in range(NCORES):
        in_maps.append(
            {
                "xs_t": xs_t,
                "w1": W1,
                "w2": w2p,
                "dinvt": meta["dinv_ts"][c],
                "idx1": meta["idx1_ws"][c],
                "idx2": meta["idx2_ws"][c],
                "dst_w": meta["dst_ws"][c],
                "iota_r": meta["iota"],
            }
        )
    outs = st["runner"](in_maps)
    res = np.empty((N_NODES, N_CLASSES), np.float32)
    for c in range(NCORES):
        o = outs[c][0]  # [128, NT, N_CLASSES]
        res[c * NSH : (c + 1) * NSH] = o.transpose(1, 0, 2).reshape(NSHP, N_CLASSES)[
            :NSH
        ]
    return res


# revision 45
# speedup vs baseline: 1.1251x; 1.1251x over previous
import sys

sys.path.insert(0, "/opt/trn_rl_repo")

import hashlib

import ml_dtypes
import numpy as np

import concourse.bass as bass
import concourse.mybir as mybir
import concourse.tile as tile
from concourse.library_config import mlp
from concourse.vector_clock import ScopedClock

dt = mybir.dt
AF = mybir.ActivationFunctionType
ALU = mybir.AluOpType

N_NODES = 100000
F_IN = 128
N_CLASSES = 40
NCORES = 8
NSH = 12500
NT = 98
NSHP = NT * 128  # 12544
NTG = NCORES * NT  # 784 global node tiles
QROWS = 2 * NSHP  # 25088 rows per int16-indexable quarter (= 2 cores)
NQ = 4
KCAP = 8  # max 128-idx chunks per dma_gather call (SWDGE ring capacity)
SINGLE_PACKET = False  # spread gather descriptors across packets


class PatchedTileContext(tile.TileContext):
    # walrus CoreV3 codegen accepts at most 1 sem wait on most instruction
    # structs; spread the final-drain waits over 1-wait nops.
    def _drain_and_barrier(self, tick_clock, wait_clock):
        collector = self.nc.sync.nop(nofuse=True)
        wait_clock.add_sem_waits(
            collector.ins, ScopedClock({None: tick_clock.global_clock})
        )
        si = collector.ins.sync_info
        waits = list(si.on_wait) if si and si.on_wait else []
        if len(waits) > 1:
            si.on_wait = waits[:1]
            for w in waits[1:]:
                extra = self.nc.sync.nop(nofuse=True)
                extra.ins.sync_info = mybir.SyncInfo(on_wait=[w], on_update=[])
        self.nc.sync.drain()
        self.nc.all_engine_barrier()
        assert self.sems is not None
        popped = self.nc._tile_sem_poison_stack.pop()
        assert popped is self._sem_poison
        self.nc.clear_and_free_semaphores(list(self.sems.allocated().values()))
        self.nc.all_engine_barrier()


def _split_excess_waits(nc, max_waits=1):
    # Same walrus limit for ordinary instructions: move excess sem waits onto
    # single-wait carrier instructions on the same engine, inserted just
    # before (per-engine order makes the stall equivalent).
    cnt = 0
    for f in nc.m.functions:
        for bb in f.blocks:
            insns = bb.instructions
            i = 0
            while i < len(insns):
                ins = insns[i]
                si = getattr(ins, "sync_info", None)
                waits = list(si.on_wait) if si is not None and si.on_wait else []
                if len(waits) > max_waits:
                    si.on_wait = waits[:1]
                    for w in waits[1:]:
                        if ins.engine == mybir.EngineType.Pool:
                            nop = mybir.InstEventSemaphore(
                                name=f"waitsplit_{cnt}", ins=[], outs=[]
                            )
                        else:
                            nop = mybir.InstNoOp(
                                name=f"waitsplit_{cnt}", ins=[], outs=[]
                            )
                        cnt += 1
                        nop.engine = ins.engine
                        nop.sync_info = mybir.SyncInfo(on_wait=[w], on_update=[])
                        insns.insert(i, nop)
                        i += 1
                i += 1
    return cnt


def _preprocess(edge_index):
    """Structure-only preprocessing (depends on edge_index, not on x/W).

    Layer tables are stored partition-major: row of node (g, p) lives at
    flat index p*196 + (g % 196) within its quarter's table; the AllGather
    table row for (core c, tile t, slot p) is ((c*128 + p)*NT + t) so the
    kernel can publish [128, NT, 128] tiles with identity-layout DMAs.
    """
    src = np.asarray(edge_index[0], dtype=np.int64)
    dst = np.asarray(edge_index[1], dtype=np.int64)
    deg = np.bincount(dst, minlength=N_NODES).astype(np.float32) + 1.0
    dinv = (1.0 / np.sqrt(deg)).astype(np.float32)
    loops = np.arange(N_NODES, dtype=np.int64)
    src = np.concatenate([src, loops])
    dst = np.concatenate([dst, loops])

    # per-edge source coordinates (global tile g, slot ps)
    s_core = src // NSH
    s_loc = src % NSH
    g = s_core * NT + (s_loc >> 7)
    ps = s_loc & 127
    q_of_edge = g // (2 * NT)
    # layer-1 table index (partition-major within quarter)
    lidx1 = ps * 196 + (g % 196)
    # layer-2 table index within quarter piece
    lidx2 = ((s_core - 2 * q_of_edge) * 128 + ps) * NT + (g % NT)

    core_of = dst // NSH
    per_core = []
    counts = np.zeros((NCORES, NT, NQ), np.int64)
    for c in range(NCORES):
        m = core_of == c
        ed = dst[m] - c * NSH
        t = ed >> 7
        slot = ed & 127
        q = q_of_edge[m]
        l1 = lidx1[m]
        l2 = lidx2[m]
        key = t * NQ + q  # sort by (t, q)
        order = np.argsort(key, kind="stable")
        per_core.append((t[order], q[order], slot[order], l1[order], l2[order]))
        np.add.at(counts[c], (t[order], q[order]), 1)

    K = np.ceil(counts / 128.0).astype(np.int64).max(axis=0)  # [NT, NQ]
    # chunk layout: for t: for q: K[t, q] chunks (tile's chunks contiguous)
    chunk_base = np.zeros((NT, NQ), np.int64)
    off = 0
    for t in range(NT):
        for q in range(NQ):
            chunk_base[t, q] = off
            off += K[t, q]
    nch_tot = off
    nchf = int(K.sum(axis=1).max())
    tot = nch_tot * 128

    idx1_ws, idx2_ws, dst_ws = [], [], []
    for c in range(NCORES):
        t, q, slot, l1, l2 = per_core[c]
        # rank of each edge within its (t, q) bucket (edges sorted by (t, q))
        key = t * NQ + q
        uniq, first, inv = np.unique(key, return_index=True, return_inverse=True)
        rank = np.arange(len(t)) - first[inv]
        pos = chunk_base[t, q] * 128 + rank
        idx1_flat = np.zeros(tot, np.int16)
        idx2_flat = np.zeros(tot, np.int16)
        dst_flat = np.full(tot, 999.0, np.float32)
        idx1_flat[pos] = l1.astype(np.int16)
        idx2_flat[pos] = l2.astype(np.int16)
        dst_flat[pos] = slot.astype(np.float32)
        # [128, tot//16]: 16-row wrap replicated 8x (one copy per gpsimd core)
        idx1_ws.append(
            np.ascontiguousarray(np.tile(idx1_flat.reshape(tot // 16, 16).T, (8, 1)))
        )
        idx2_ws.append(
            np.ascontiguousarray(np.tile(idx2_flat.reshape(tot // 16, 16).T, (8, 1)))
        )
        dst_ws.append(
            np.ascontiguousarray(
                dst_flat.reshape(nch_tot, 128).T
            )
        )

    iota = np.tile(np.arange(128, dtype=np.float32)[None, :], (128, nchf))
    iota = np.ascontiguousarray(
        iota.reshape(128, nchf, 128)
    )

    # dinv lookups: dinva[p, g] = dinv of node (g, p) (0 on padding)
    gg, pp = np.meshgrid(np.arange(NTG), np.arange(128), indexing="xy")
    node = (gg // NT) * NSH + (gg % NT) * 128 + pp  # [128, NTG]
    valid = ((gg % NT) * 128 + pp) < NSH
    dinva = np.where(valid, dinv[np.minimum(node, N_NODES - 1)], 0.0).astype(
        np.float32
    )
    dinv_ts = [np.ascontiguousarray(dinva[:, c * NT : (c + 1) * NT]) for c in range(NCORES)]

    return dict(
        K=K,
        chunk_base=chunk_base,
        nch_tot=nch_tot,
        nchf=nchf,
        tot=tot,
        idx1_ws=idx1_ws,
        idx2_ws=idx2_ws,
        dst_ws=dst_ws,
        iota=iota,
        dinva=np.ascontiguousarray(dinva),
        dinv_ts=dinv_ts,
    )


def _build(meta, ablate=()):
    K = meta["K"]
    chunk_base = meta["chunk_base"]
    nch_tot = meta["nch_tot"]
    nchf = meta["nchf"]

    nc = bass.Bass(num_devices=NCORES, num_swdge_queues=4)
    xs_in = nc.dram_tensor(
        "xs_t", [NQ, QROWS, F_IN], dt.bfloat16, kind="ExternalInput"
    )
    # inputs are packed by dtype: per-execute cost is ~45us per input buffer
    b16_in = nc.dram_tensor(
        "b16", [128, nch_tot * 16], dt.int16, kind="ExternalInput"
    )
    b32cols = F_IN + 64 + NT + nch_tot + nchf * 128
    b32_in = nc.dram_tensor("b32", [128, b32cols], dt.float32, kind="ExternalInput")
    out_t = nc.dram_tensor(
        "out_s", [128, NT, N_CLASSES], dt.float32, kind="ExternalOutput"
    )

    with PatchedTileContext(nc) as tc:
        with (
            tc.tile_pool(name="sbuf", bufs=1) as pool,
            tc.tile_pool(name="psum", bufs=1, space="PSUM") as psum,
            tc.tile_pool(name="dram", bufs=1, space="DRAM") as dram,
        ):
            w1_t = pool.tile([F_IN, F_IN], dt.float32)
            w2_t = pool.tile([F_IN, 64], dt.float32)
            dinvt_t = pool.tile([128, NT], dt.float32)
            # one idx buffer, re-filled between layers (WAR dep handled by Tile)
            idx_t = pool.tile([128, nch_tot * 8], dt.int16)
            dst_t = pool.tile([128, nch_tot], dt.float32)
            iota_t = pool.tile([128, nchf, 128], dt.float32)
            z_all = pool.tile([128, NT, N_CLASSES], dt.float32)
            zs_all = pool.tile([128, NT, N_CLASSES], dt.float32)
            nc.gpsimd.load_library(mlp)
            o1 = F_IN
            o2 = o1 + 64
            o3 = o2 + NT
            o4 = o3 + nch_tot
            for d_ap, s_ap in [
                (w1_t, b32_in[:, 0:o1]),
                (w2_t, b32_in[:, o1:o2]),
                (dinvt_t, b32_in[:, o2:o3]),
                (dst_t, b32_in[:, o3:o4]),
                (iota_t, b32_in[:, o4:b32cols]),
                (idx_t, b16_in[:, 0 : nch_tot * 8]),
            ]:
                nc.sync.dma_start(d_ap[:], s_ap[:])

            bounce = dram.tile([128, NT, F_IN], dt.bfloat16)
            table2 = dram.tile(
                [NCORES * 128, NT, F_IN], dt.bfloat16, addr_space="Shared"
            )

            # gpsimd registers are scarce: one per distinct idx count, reused
            reg_cache = {}

            def nreg(v):
                if v not in reg_cache:
                    reg_cache[v] = nc.gpsimd.to_reg(v)
                return reg_cache[v]

            def aggregate(idx_t, tables, scope, tail):
                """Per dst tile t: acc[t] = sum over edges of table[src]^T,
                accumulated in PSUM across all 4 quarters, then tail(t, accs).

                tables[q] is an AP-able of shape [QROWS, 128] (bf16 rows).
                """
                _sid, _ = nc.enter_named_scope(scope, False)
                for t in range(NT):
                    o = int(chunk_base[t, 0])
                    ntot = int(K[t].sum())
                    assert ntot > 0
                    sel_t = pool.tile(
                        [128, nchf, 128], dt.bfloat16, name="sel", bufs=2
                    )
                    nc.vector.tensor_tensor(
                        out=sel_t[:, 0:ntot, :],
                        in0=dst_t[:, o : o + ntot].to_broadcast([128, ntot, 128]),
                        in1=iota_t[:, 0:ntot, :],
                        op=ALU.is_equal,
                    )
                    acc = psum.tile([128, 128], dt.float32, name="acc", bufs=2)
                    done = 0
                    for q in range(NQ):
                        kb = int(K[t, q])
                        ob = int(chunk_base[t, q])
                        for p0 in range(0, kb, KCAP):
                            kp = min(KCAP, kb - p0)
                            g_t = pool.tile(
                                [128, KCAP, 128], dt.bfloat16, name=f"g{q}", bufs=3
                            )
                            if "gather" not in ablate:
                                nc.gpsimd.dma_gather(
                                    g_t[:, 0:kp, :],
                                    tables[q],
                                    idx_t[:, (ob + p0) * 8 : (ob + p0 + kp) * 8],
                                    num_idxs=kp * 128,
                                    num_idxs_reg=nreg(kp * 128),
                                    elem_size=F_IN,
                                    queue_num=q,
                                    single_packet=SINGLE_PACKET,
                                )
                            elif "pe" not in ablate:
                                nc.vector.memset(g_t[:, 0:kp, :], 0.0)
                            cb = ob + p0 - o
                            for k in range(kp):
                                if "pe" in ablate:
                                    if done == 0:
                                        nc.vector.memset(acc[:], 0.0)
                                    done = ntot
                                    break
                                nc.tensor.matmul(
                                    acc[:],
                                    lhsT=g_t[:, k, :],
                                    rhs=sel_t[:, cb + k, :],
                                    start=(done == 0),
                                    stop=(done == ntot - 1),
                                )
                                done += 1
                    accs = pool.tile([128, 128], dt.float32, name="accs", bufs=3)
                    nc.scalar.copy(accs[:], acc[:])
                    tail(t, accs)
                nc.leave_named_scope(scope, _sid, False)

            # ---- layer 1
            def l1_tail(t, accs):
                h1p = psum.tile([128, 128], dt.float32, name="mm", bufs=2)
                nc.tensor.matmul(
                    h1p[:], lhsT=accs[:], rhs=w1_t[:], start=True, stop=True
                )
                a1 = pool.tile([128, 128], dt.float32, name="a1", bufs=2)
                nc.scalar.activation(
                    a1[:], h1p[:], AF.Relu, bias=0.0, scale=dinvt_t[:, t : t + 1]
                )
                p16 = pool.tile([128, 128], dt.bfloat16, name="p16", bufs=3)
                nc.scalar.activation(
                    p16[:], a1[:], AF.Copy, bias=0.0, scale=dinvt_t[:, t : t + 1]
                )
                nc.sync.dma_start(bounce[:, t, :], p16[:])

            aggregate(idx_t, [xs_in[q] for q in range(NQ)], "agg1", l1_tail)

            _sidG, _ = nc.enter_named_scope("ag", False)
            if "cc" not in ablate:
                nc.gpsimd.collective_compute(
                    "AllGather",
                    ALU.bypass,
                    replica_groups=[list(range(NCORES))],
                    ins=[bounce.opt()],
                    outs=[table2.opt()],
                )
            nc.leave_named_scope("ag", _sidG, False)

            # ---- layer 2
            nc.sync.dma_start(idx_t[:], b16_in[:, nch_tot * 8 : nch_tot * 16])
            t2 = table2[:].flatten_outer_dims()

            def l2_tail(t, accs):
                zp = psum.tile([128, 64], dt.float32, name="zp", bufs=2)
                nc.tensor.matmul(
                    zp[:], lhsT=accs[:], rhs=w2_t[:], start=True, stop=True
                )
                nc.scalar.activation(
                    z_all[:, t, :],
                    zp[:, 0:N_CLASSES],
                    AF.Copy,
                    bias=0.0,
                    scale=dinvt_t[:, t : t + 1],
                )

            aggregate(
                idx_t,
                [t2[q * QROWS : (q + 1) * QROWS, :] for q in range(NQ)],
                "agg2",
                l2_tail,
            )
            _sid2, _ = nc.enter_named_scope("l2tail", False)
            # batched log_softmax over classes
            mx = pool.tile([128, NT, 1], dt.float32, name="mx")
            nc.vector.tensor_reduce(mx[:], z_all[:], mybir.AxisListType.X, ALU.max)
            nc.vector.tensor_tensor(
                out=zs_all[:],
                in0=z_all[:],
                in1=mx[:, :, 0].to_broadcast([128, NT, N_CLASSES]),
                op=ALU.subtract,
            )
            nc.scalar.activation(z_all[:], zs_all[:], AF.Exp, bias=0.0, scale=1.0)
            sm = pool.tile([128, NT, 1], dt.float32, name="sm")
            nc.vector.tensor_reduce(sm[:], z_all[:], mybir.AxisListType.X, ALU.add)
            ls = pool.tile([128, NT, 1], dt.float32, name="ls")
            nc.scalar.activation(ls[:], sm[:], AF.Ln, bias=0.0, scale=1.0)
            nc.vector.tensor_tensor(
                out=z_all[:],
                in0=zs_all[:],
                in1=ls[:, :, 0].to_broadcast([128, NT, N_CLASSES]),
                op=ALU.subtract,
            )
            nc.sync.dma_start(out_t[:], z_all[:])
            nc.leave_named_scope("l2tail", _sid2, False)

    _split_excess_waits(nc)
    mybir.codegen_inst_isa_subclasses(nc)
    return nc


def _make_runner(nc):
    import jax
    from jax.sharding import Mesh, PartitionSpec

    try:
        from jax.experimental.shard_map import shard_map
    except ImportError:
        from jax.shard_map import shard_map

    from concourse.bass2jax import (
        _bass_exec_p,
        install_neuronx_cc_hook,
        partition_id_tensor,
    )

    install_neuronx_cc_hook()
    assert nc.dbg_addr is None
    partition_name = nc.partition_id_tensor.name if nc.partition_id_tensor else None

    in_names, out_names, out_avals = [], [], []
    for alloc in nc.m.functions[0].allocations:
        if not isinstance(alloc, mybir.MemoryLocationSet):
            continue
        name = alloc.memorylocations[0].name
        if alloc.kind == "ExternalInput":
            if name != partition_name:
                in_names.append(name)
        elif alloc.kind == "ExternalOutput":
            out_names.append(name)
            shape = tuple(alloc.tensor_shape)
            dtype = mybir.dt.np(alloc.dtype)
            out_avals.append(jax.core.ShapedArray(shape, dtype))
    n_params = len(in_names)
    n_outs = len(out_avals)
    all_names = in_names + out_names
    if partition_name is not None:
        all_names = all_names + [partition_name]
    donate = tuple(range(n_params, n_params + n_outs))

    def _body(*args):
        operands = list(args)
        if partition_name is not None:
            operands.append(partition_id_tensor())
        outs = _bass_exec_p.bind(
            *operands,
            out_avals=tuple(out_avals),
            in_names=tuple(all_names),
            out_names=tuple(out_names),
            lowering_input_output_aliases=(),
            sim_require_finite=True,
            sim_require_nnan=True,
            nc=nc,
        )
        return tuple(outs)

    devices = jax.devices()[:NCORES]
    mesh = Mesh(np.asarray(devices), ("core",))
    in_specs = (PartitionSpec("core"),) * (n_params + n_outs)
    out_specs = (PartitionSpec("core"),) * n_outs
    sharded = jax.jit(
        shard_map(
            _body, mesh=mesh, in_specs=in_specs, out_specs=out_specs, check_rep=False
        ),
        donate_argnums=donate,
        keep_unused=True,
    )

    state = {"dev_in": None, "dev_key": None}

    def run(in_maps):
        per_core = [[np.asarray(m[name]) for name in in_names] for m in in_maps]
        concat_in = [
            np.concatenate([per_core[c][i] for c in range(NCORES)], axis=0)
            for i in range(n_params)
        ]
        hkey = hashlib.sha1()
        for a in concat_in:
            hkey.update(a.tobytes())
        hkey = hkey.hexdigest()
        if state["dev_key"] != hkey:
            from jax.sharding import NamedSharding

            state["dev_in"] = [
                jax.device_put(a, NamedSharding(mesh, PartitionSpec("core")))
                for a in concat_in
            ]
            state["dev_key"] = hkey
        concat_zeros = [
            np.zeros((NCORES * a.shape[0], *a.shape[1:]), a.dtype) for a in out_avals
        ]
        out_arrs = sharded(*state["dev_in"], *concat_zeros)
        jax.block_until_ready(out_arrs)
        return [
            [
                np.asarray(out_arrs[i]).reshape(NCORES, *out_avals[i].shape)[c]
                for i in range(n_outs)
            ]
            for c in range(NCORES)
        ]

    run.sharded = sharded
    run.state = state
    run.mesh = mesh
    run.out_avals = out_avals
    run.body = _body
    run.n_params = n_params
    return run


_CACHE = {}


def kernel(**inputs):
    x = np.asarray(inputs["x"], np.float32)
    ei = np.asarray(inputs["edge_index"])
    W1 = np.asarray(inputs["W1"], np.float32)
    W2 = np.asarray(inputs["W2"], np.float32)
    b1 = np.asarray(inputs["b1"], np.float32)
    b2 = np.asarray(inputs["b2"], np.float32)
    assert not b1.any() and not b2.any(), "nonzero biases not supported"

    key = hashlib.sha1(ei.tobytes()).hexdigest()
    st = _CACHE.get(key)
    if st is None:
        meta = _preprocess(ei)
        nc = _build(meta)
        runner = _make_runner(nc)
        st = {"meta": meta, "runner": runner}
        _CACHE.clear()
        _CACHE[key] = st
    meta = st["meta"]

    # layer-1 gather table: xs[q, p*196 + gl, :] = dinv[n] * x[n] (bf16)
    # for node n of global tile g = q*196 + gl, slot p (zeros on padding)
    xpad = np.zeros((NCORES, NT * 128, F_IN), np.float32)
    for c in range(NCORES):
        xpad[c, :NSH] = meta["dinva"].T.reshape(NTG * 128, 1)[
            c * NSHP : c * NSHP + NSH
        ] * x[c * NSH : (c + 1) * NSH]
    xs_t = np.ascontiguousarray(
        xpad.reshape(NQ, 196, 128, F_IN)
        .transpose(0, 2, 1, 3)
        .reshape(NQ, QROWS, F_IN)
        .astype(ml_dtypes.bfloat16)
    )

    w2p = np.zeros((F_IN, 64), np.float32)
    w2p[:, :N_CLASSES] = W2
    nchf = meta["nchf"]
    in_maps = []
    for c in range(NCORES):
        b32 = np.ascontiguousarray(
            np.concatenate(
                [
                    W1,
                    w2p,
                    meta["dinv_ts"][c],
                    meta["dst_ws"][c],
                    meta["iota"].reshape(128, nchf * 128),
                ],
                axis=1,
            )
        )
        b16 = np.ascontiguousarray(
            np.concatenate([meta["idx1_ws"][c], meta["idx2_ws"][c]], axis=1)
        )
        in_maps.append({"xs_t": xs_t, "b16": b16, "b32": b32})
    outs = st["runner"](in_maps)
    res = np.empty((N_NODES, N_CLASSES), np.float32)
    for c in range(NCORES):
        o = outs[c][0]  # [128, NT, N_CLASSES]
        res[c * NSH : (c + 1) * NSH] = o.transpose(1, 0, 2).reshape(NSHP, N_CLASSES)[
            :NSH
        ]
    return res
